# revision 24
# baseline (speedup 1.0000x reference)
"""Causal self-attention (B=4, S=2048, D=1024, H=16, rope) on 8 trn2 cores.

Sharding: batch x head-half. Core c handles batch b=c//2 and heads
hh*8..hh*8+7 where hh=c%2. Each core computes its 8 heads' attention over its
batch and a partial output projection; the host sums the two partials per
batch.

v2: fully pipelined head-pair schedule. The 8 local heads form 4 pairs
(pair p = local dims [128p, 128p+128)). Per pair: q/k projection (bf16
inputs, f32r slabs), then attention over 4 q-quarters of 512 with the two
heads' QK^T matmuls packed onto disjoint PE row halves (tile_position via
base partitions 0/64), one exp per (quarter, k-tile) covering both heads,
PV in bf16 with a ones-column producing softmax denominators, and per-
quarter normalization via reciprocal_approx_fast + gpsimd
partition_broadcast. Projection of pair p+1 is interleaved into pair p's
attention so the PE stays busy while ACT streams exps; the output
projection is interleaved into pair 3's attention.
"""

import numpy as np

B, S, D, H, DK = 4, 2048, 1024, 16, 64
THETA = 10000.0
N_CORES = 8
NKT = S // 128        # k tiles
NP = 4                # head pairs per core
NQ = 4                # q quarters of 512

_prog_cache = {}


def _apply_walrus_wait_workarounds():
    """This container's walrus rejects any TPB instruction with more than one
    sync wait. Patch the Tile kernel-tail drain to emit a chain of single-wait
    drains, and provide a post-pass that hoists excess waits onto NoOps."""
    import concourse.mybir as mybir
    import concourse.tile as tile_mod
    from concourse.vector_clock import ScopedClock

    def _drain_and_barrier(self, tick_clock, wait_clock):
        nc = self.nc
        drain_inst = nc.sync.drain()
        wait_clock.add_sem_waits(
            drain_inst.ins, ScopedClock({None: tick_clock.global_clock}))
        waits = list(drain_inst.ins.sync_info.on_wait)
        if len(waits) > 1:
            si = drain_inst.ins.sync_info
            si.on_wait = waits[:1]
            drain_inst.ins.sync_info = si
            for i in range(1, len(waits)):
                d2 = nc.sync.drain()
                d2.ins.sync_info = mybir.SyncInfo(
                    on_wait=waits[i:i + 1], on_update=[])
        nc.all_engine_barrier()
        popped = nc._tile_sem_poison_stack.pop()
        assert popped is self._sem_poison
        nc.clear_and_free_semaphores(list(self.sems.allocated().values()))
        nc.all_engine_barrier()

    tile_mod.TileContext._drain_and_barrier = _drain_and_barrier


def _split_waits(nc):
    import concourse.mybir as mybir
    engines = {mybir.EngineType.PE, mybir.EngineType.DVE, mybir.EngineType.SP,
               mybir.EngineType.Activation, mybir.EngineType.Pool}
    for f in nc.m.functions:
        for bb in f.blocks:
            out = []
            changed = False
            for ins in bb.instructions:
                si = ins.sync_info
                if si is not None and len(si.on_wait) > 1 and ins.engine in engines:
                    waits = list(si.on_wait)
                    for i in range(len(waits) - 1):
                        out.append(mybir.InstNoOp(
                            name=f"{ins.name}-waitsplit-{i}",
                            sync_info=mybir.SyncInfo(
                                on_wait=waits[i:i + 1], on_update=[]),
                            bass_nofuse=True, engine=ins.engine))
                    ins.sync_info = mybir.SyncInfo(
                        on_wait=waits[-1:], on_update=list(si.on_update))
                    changed = True
                out.append(ins)
            if changed:
                bb.instructions = out


def _pair_swap_mask():
    mask = []
    for j in range(16):
        mask += [2 * j + 1, 2 * j]
    return mask


def _build_program():
    _apply_walrus_wait_workarounds()
    import concourse.bass as bass
    import concourse.mybir as mybir
    import concourse.tile as tile
    from concourse import library_config
    from concourse.masks import make_identity
    from contextlib import ExitStack

    F32 = mybir.dt.float32
    F32R = mybir.dt.float32r
    BF16 = mybir.dt.bfloat16
    AF = mybir.ActivationFunctionType

    nc = bass.Bass()
    xb = nc.declare_dram_parameter("xb", [S, D], F32, isOutput=False)
    wqt = nc.declare_dram_parameter("wqt", [D, 512], F32, isOutput=False)
    wkt = nc.declare_dram_parameter("wkt", [D, 512], F32, isOutput=False)
    wvt = nc.declare_dram_parameter("wvt", [D, 512], F32, isOutput=False)
    wot = nc.declare_dram_parameter("wot", [512, D], F32, isOutput=False)
    cost = nc.declare_dram_parameter("cost", [128, S], F32, isOutput=False)
    sint2 = nc.declare_dram_parameter("sint2", [128, S], F32, isOutput=False)
    esel2 = nc.declare_dram_parameter("esel2", [8, 4, 128], F32, isOutput=False)
    y = nc.declare_dram_parameter("y", [S, D], F32, isOutput=True)

    swap_mask = _pair_swap_mask()

    with tile.TileContext(nc) as tc, ExitStack() as ctx:
        singles = ctx.enter_context(tc.tile_pool(name="singles", bufs=1))
        ident = singles.tile([128, 128], F32)
        make_identity(nc, ident)

        # persistent tensors
        xtc = singles.tile([128, 8, S], BF16, tag="xtc")       # x^T, bf16
        qslabs = [singles.tile([128, S], F32R, tag=f"qsl{p}", name=f"qsl{p}")
                  for p in range(NP)]                          # doubles as attn out
        vslab = singles.tile([128, NKT, 8, 65], BF16, tag="vslab")
        cosc = singles.tile([128, S], F32, tag="cosc")
        sinc = singles.tile([128, S], F32, tag="sinc")
        wvr = singles.tile([128, 8, 512], BF16, tag="wvr")
        wor = singles.tile([128, 4, D], F32R, tag="wor")
        selr = singles.tile([8, 4, 128], F32R, tag="selr")
        sums = [singles.tile([8, 512], F32, tag=f"sums{p}", name=f"sums{p}")
                for p in range(NP)]

        # pools
        xpool = ctx.enter_context(tc.tile_pool(name="xpool", bufs=4))
        wst = ctx.enter_context(tc.tile_pool(name="wst", bufs=2))
        wrq = ctx.enter_context(tc.tile_pool(name="wrq", bufs=2))
        wrk = ctx.enter_context(tc.tile_pool(name="wrk", bufs=2))
        kpool = ctx.enter_context(tc.tile_pool(name="kpool", bufs=2))
        ptpool = ctx.enter_context(tc.tile_pool(name="ptpool", bufs=4))
        tshp = ctx.enter_context(tc.tile_pool(name="tshp", bufs=2))
        nrm = ctx.enter_context(tc.tile_pool(name="nrm", bufs=1))
        ysb = ctx.enter_context(tc.tile_pool(name="ysb", bufs=2))
        ppp = ctx.enter_context(tc.tile_pool(name="ppp", bufs=2, space="PSUM"))
        pss = ctx.enter_context(tc.tile_pool(name="pss", bufs=2, space="PSUM"))
        posp = ctx.enter_context(tc.tile_pool(name="posp", bufs=1, space="PSUM"))

        nc.sync.dma_start(out=cosc, in_=cost[:, :])
        nc.sync.dma_start(out=sinc, in_=sint2[:, :])

        # ---- weight staging helpers ------------------------------------
        def stage_wv():
            wv_r = wvt.rearrange("(ic p) o -> p ic o", p=128)
            for ic in range(8):
                st = wst.tile([128, 1024], F32, tag="wst", name=f"wv{ic}")
                nc.sync.dma_start(out=st[:, 0:512], in_=wv_r[:, ic, :])
                nc.vector.tensor_copy(wvr[:, ic, :], st[:, 0:512])

        def stage_wqk(p):
            """DMA + cast pair p's q/k weight slices -> [128, 8, 128] bf16."""
            tiles = {}
            for name, src, pool in (("q", wqt, wrq), ("k", wkt, wrk)):
                src_r = src.rearrange("(ic pp) o -> pp ic o", pp=128)
                st = wst.tile([128, 1024], F32, tag="wst", name=f"w{name}st{p}")
                st_v = st.rearrange("pp (ic o) -> pp ic o", ic=8)
                nc.sync.dma_start(
                    out=st_v, in_=src_r[:, :, p * 128:(p + 1) * 128])
                wr = pool.tile([128, 8, 128], BF16, tag=f"wr{name}",
                               name=f"wr{name}{p}")
                nc.vector.tensor_copy(wr, st_v)
                tiles[name] = wr
            return tiles

        def stage_wo():
            wot_r = wot.rearrange("(ic p) o -> p ic o", p=128)
            for ic in range(4):
                st = wst.tile([128, 1024], F32, tag="wst", name=f"wo{ic}")
                nc.sync.dma_start(out=st, in_=wot_r[:, ic, :])
                nc.vector.tensor_copy(wor[:, ic, :], st)

        # ---- phase helpers ---------------------------------------------
        def emit_xt_chunk(sc4):
            """Transpose x s-chunk sc4 into xtc (bf16) + project v for it."""
            xts = []
            for ssub in range(4):
                xt = xpool.tile([128, D], F32, tag="x",
                                name=f"x{sc4}_{ssub}")
                s0 = sc4 * 512 + ssub * 128
                nc.sync.dma_start(out=xt, in_=xb[s0:s0 + 128, :])
                xts.append(xt)
            for ic in range(8):
                ptr = ppp.tile([128, 512], F32, tag="pp", name="ptr")
                for ssub in range(4):
                    nc.tensor.transpose(
                        ptr[:, ssub * 128:(ssub + 1) * 128],
                        xts[ssub][:, ic * 128:(ic + 1) * 128], ident)
                if sc4 == 0:
                    nc.scalar.copy(
                        out=xtc[:, ic, sc4 * 512:(sc4 + 1) * 512], in_=ptr)
                else:
                    nc.vector.tensor_copy(
                        xtc[:, ic, sc4 * 512:(sc4 + 1) * 512], ptr)
            # v projection for this s-chunk (all 8 heads)
            for ssub in range(4):
                pv = ppp.tile([128, 512], F32, tag="pp", name="pv")
                for ic in range(8):
                    nc.tensor.matmul(
                        pv,
                        lhsT=xtc[:, ic, sc4 * 512 + ssub * 128:
                                 sc4 * 512 + (ssub + 1) * 128],
                        rhs=wvr[:, ic, :],
                        start=(ic == 0), stop=(ic == 7))
                kt = sc4 * 4 + ssub
                nc.vector.tensor_copy(
                    vslab[:, kt, :, 0:64],
                    pv.rearrange("p (h dk) -> p h dk", h=8))

        def proj_qk_group(p, wr_tiles, t, sc4):
            """One projection group: pair p, tensor t in {q, k}, s-chunk sc4."""
            slab = qslabs[p] if t == "q" else kslab_tiles[p]
            wr = wr_tiles[t]
            ssl = slice(sc4 * 512, (sc4 + 1) * 512)
            pp = ppp.tile([128, 512], F32, tag="pp", name=f"pp{t}{p}_{sc4}")
            for ic in range(8):
                nc.tensor.matmul(
                    pp, lhsT=wr[:, ic, :], rhs=xtc[:, ic, ssl],
                    start=(ic == 0), stop=(ic == 7))
            tsh = tshp.tile([128, 512], F32, tag="tsh")
            nc.vector.stream_shuffle(tsh, pp, swap_mask)
            nc.gpsimd.tensor_mul(tsh, tsh, sinc[:, ssl])
            nc.vector.tensor_mul(slab[:, ssl], pp, cosc[:, ssl])
            nc.vector.tensor_add(slab[:, ssl], slab[:, ssl], tsh)

        kslab_tiles = {}

        def attention_quarter(p, q):
            """Attention for pair p, q-quarter q."""
            qsl0 = q * 512
            kslab = kslab_tiles[p]
            qslab = qslabs[p]
            pos = posp.tile([65, 1024], F32, tag="pos", name=f"pos{p}_{q}")
            njs = 4 * q + 4
            pending = []
            for j in range(njs):
                lo = max(0, 128 * j - 512 * q)
                w = 512 - lo
                qsl = slice(qsl0 + lo, qsl0 + 512)
                # head A scores at ps[:, 0:w] (bank 0), head B at
                # ps[:, 512:512+w] (bank 1) — each matmul stays in one bank
                ps = pss.tile([128, 1024], F32, tag="ps", name=f"ps{p}_{q}_{j}")
                nc.tensor.matmul(
                    ps[:, 0:w],
                    lhsT=kslab[0:64, j * 128:(j + 1) * 128],
                    rhs=qslab[0:64, qsl], start=True, stop=True)
                nc.tensor.matmul(
                    ps[:, 512:512 + w],
                    lhsT=kslab[64:128, j * 128:(j + 1) * 128],
                    rhs=qslab[64:128, qsl], start=True, stop=True)
                pt = ptpool.tile([128, 1024], BF16, tag="pt")
                nc.scalar.activation(out=pt[:, 0:512 + w],
                                     in_=ps[:, 0:512 + w],
                                     func=AF.Exp, scale=0.125)
                if j >= 4 * q:  # diagonal tile: mask upper triangle
                    for h in range(2):
                        nc.gpsimd.affine_select(
                            out=pt[:, h * 512:h * 512 + 128],
                            in_=pt[:, h * 512:h * 512 + 128],
                            compare_op=mybir.AluOpType.is_ge,
                            fill=0.0, base=0,
                            pattern=[[1, 128]], channel_multiplier=-1)
                pending.append((j, lo, w, pt))
                # drain PV one step behind exp
                if len(pending) > 1:
                    emit_pv(p, q, pos, *pending.pop(0), njs)
            while pending:
                emit_pv(p, q, pos, *pending.pop(0), njs)
            # ---- writeback (unnormalized; frees pos quickly) -----------
            stmp = nrm.tile([1, 1024], F32, tag="stmp")
            nc.vector.tensor_copy(stmp[0:1, 0:512], pos[64:65, 0:512])
            nc.vector.tensor_copy(stmp[0:1, 512:1024], pos[64:65, 512:1024])
            nc.sync.dma_start(out=sums[p][q:q + 1, :], in_=stmp[0:1, 0:512])
            nc.sync.dma_start(out=sums[p][4 + q:5 + q, :],
                              in_=stmp[0:1, 512:1024])
            qsl = slice(qsl0, qsl0 + 512)
            nc.vector.tensor_copy(qslab[0:64, qsl], pos[0:64, 0:512])
            nc.vector.tensor_copy(qslab[64:128, qsl], pos[0:64, 512:1024])

        def normalize_quarters(p, quarters):
            """Scale pair p's attention output by the softmax reciprocals for
            the given quarters (requires their sums rows to be final)."""
            qslab = qslabs[p]
            recs = nrm.tile([8, 512], F32, tag="recs")
            nc.vector.reciprocal(recs, sums[p])
            recr = nrm.tile([8, 512], F32R, tag="recr")
            nc.vector.tensor_copy(recr, recs)
            for q in quarters:
                pb = ppp.tile([128, 512], F32, tag="pp", name=f"pb{p}_{q}")
                nc.tensor.matmul(pb, lhsT=selr[:, q, :], rhs=recr,
                                 start=True, stop=True)
                qsl = slice(q * 512, (q + 1) * 512)
                nc.vector.tensor_mul(qslab[:, qsl], qslab[:, qsl], pb)

        def emit_pv(p, q, pos, j, lo, w, pt, njs):
            start = (j == 0)
            stop = (j == njs - 1)
            nc.tensor.matmul(
                pos[:, lo:512], lhsT=vslab[:, j, 2 * p, 0:65],
                rhs=pt[:, 0:w], start=start, stop=stop)
            nc.tensor.matmul(
                pos[:, 512 + lo:1024], lhsT=vslab[:, j, 2 * p + 1, 0:65],
                rhs=pt[:, 512:512 + w], start=start, stop=stop)

        def outproj_group(qs):
            for oh in range(2):
                py = ppp.tile([128, 512], F32, tag="pp", name=f"py{qs}_{oh}")
                for p in range(NP):
                    nc.tensor.matmul(
                        py, lhsT=qslabs[p][:, qs * 128:(qs + 1) * 128],
                        rhs=wor[:, p, oh * 512:(oh + 1) * 512],
                        start=(p == 0), stop=(p == 3))
                yt = ysb.tile([128, 512], F32, tag="yt", name=f"yt{qs}_{oh}")
                nc.scalar.copy(out=yt, in_=py)
                nc.sync.dma_start(
                    out=y[qs * 128:(qs + 1) * 128, oh * 512:(oh + 1) * 512],
                    in_=yt)

        # ================= emission =====================================
        selst = nrm.tile([8, 4, 128], F32, tag="selst")
        nc.sync.dma_start(out=selst, in_=esel2[:])
        nc.vector.tensor_copy(selr, selst)
        nc.vector.memset(vslab[:, :, :, 64:65], 1.0)
        nc.vector.memset(sums[NP - 1], 1.0)
        stage_wv()

        # global queue of projection groups, popped ahead of the attention
        # quarters that need them
        proj_plan = [(p, t, sc4) for p in range(NP)
                     for sc4 in range(4) for t in ("q", "k")]
        wr_by_pair = {}
        emitted = [0]

        def pop_proj(target):
            while emitted[0] < min(target, len(proj_plan)):
                p, t, sc4 = proj_plan[emitted[0]]
                if p not in wr_by_pair:
                    wr_by_pair[p] = stage_wqk(p)
                    kslab_tiles[p] = kpool.tile(
                        [128, S], F32R, tag="ks", name=f"ks{p}")
                proj_qk_group(p, wr_by_pair[p], t, sc4)
                emitted[0] += 1

        emit_xt_chunk(0)
        pop_proj(2)
        for sc4 in range(1, 4):
            emit_xt_chunk(sc4)
            pop_proj(2 * sc4 + 2)

        for p in range(NP):
            for q in range(NQ):
                # stay 2 groups ahead of what pair p quarter q needs
                pop_proj(p * 8 + 2 * (q + 1) + 2)
                if p == 2 and q == 3:
                    stage_wo()
                if p == NP - 1 and q > 0:
                    # normalize + output-project the previous quarter while
                    # this quarter's attention runs
                    normalize_quarters(p, [q - 1])
                    for qs in range((q - 1) * 4, q * 4):
                        outproj_group(qs)
                attention_quarter(p, q)
            if p < NP - 1:
                normalize_quarters(p, range(NQ))
        normalize_quarters(NP - 1, [NQ - 1])
        for qs in range(12, 16):
            outproj_group(qs)

    _split_waits(nc)
    return nc


def _host_inputs(x, wq, wk, wv, wo, token_positions):
    pos = np.asarray(token_positions).astype(np.float64)
    ex = np.arange(0, DK, 2, dtype=np.float64) / DK
    freq = 1.0 / (THETA ** ex)
    f = pos[:, None] * freq[None, :]                       # [S, DK/2]
    cos = np.repeat(np.cos(f), 2, axis=1).astype(np.float32)   # [S, DK]
    sin = np.repeat(np.sin(f), 2, axis=1).astype(np.float32)
    cosT = np.ascontiguousarray(cos.T)                     # [DK, S]
    sinT = np.ascontiguousarray(sin.T)
    sgn = np.where(np.arange(DK) % 2 == 0, -1.0, 1.0).astype(np.float32)
    sinT2 = sinT * sgn[:, None]
    costile = np.tile(cosT, (2, 1))                        # [128, S]
    sintile = np.tile(sinT2, (2, 1))

    wqT = np.ascontiguousarray(wq.T)
    wkT = np.ascontiguousarray(wk.T)
    wvT = np.ascontiguousarray(wv.T)
    woT = np.ascontiguousarray(wo.T)

    # selector for the per-(quarter) reciprocal broadcast matmul:
    # pb[m] for quarter q picks sums row q (m<64, head A) or 4+q (head B)
    esel2 = np.zeros((8, 4, 128), np.float32)
    for q in range(4):
        esel2[q, q, 0:64] = 1.0
        esel2[4 + q, q, 64:128] = 1.0

    in_maps = []
    for core in range(N_CORES):
        b, hh = core // 2, core % 2
        osl = slice(hh * 512, (hh + 1) * 512)
        in_maps.append({
            "xb": np.ascontiguousarray(x[b]),
            "wqt": np.ascontiguousarray(wqT[:, osl]),
            "wkt": np.ascontiguousarray(wkT[:, osl]),
            "wvt": np.ascontiguousarray(wvT[:, osl]),
            "wot": np.ascontiguousarray(woT[osl, :]),
            "cost": costile,
            "sint2": sintile,
            "esel2": esel2,
        })
    return in_maps


def run_sharded(x, wq, wk, wv, wo, token_positions, trace=False):
    from concourse.bass_utils import run_bass_kernel_spmd
    if "nc" not in _prog_cache:
        _prog_cache["nc"] = _build_program()
    nc = _prog_cache["nc"]
    in_maps = _host_inputs(x, wq, wk, wv, wo, token_positions)
    res = run_bass_kernel_spmd(nc, in_maps, list(range(N_CORES)), trace=trace)
    out = np.empty((B, S, D), np.float32)
    for b in range(B):
        out[b] = res.results[2 * b]["y"] + res.results[2 * b + 1]["y"]
    return out, res


def kernel(x, wq, wk, wv, wo, token_positions):
    x = np.asarray(x, dtype=np.float32)
    out, _ = run_sharded(
        x, np.asarray(wq, np.float32), np.asarray(wk, np.float32),
        np.asarray(wv, np.float32), np.asarray(wo, np.float32),
        np.asarray(token_positions))
    return out


# revision 25
# speedup vs baseline: 1.0737x; 1.0737x over previous
"""Causal self-attention (B=4, S=2048, D=1024, H=16, rope) on 8 trn2 cores.

Sharding: batch x head-half. Core c handles batch b=c//2 and heads
hh*8..hh*8+7 where hh=c%2. Each core computes its 8 heads' attention over its
batch and a partial output projection; the host sums the two partials per
batch.

v2: fully pipelined head-pair schedule. The 8 local heads form 4 pairs
(pair p = local dims [128p, 128p+128)). Per pair: q/k projection (bf16
inputs, f32r slabs), then attention over 4 q-quarters of 512 with the two
heads' QK^T matmuls packed onto disjoint PE row halves (tile_position via
base partitions 0/64), one exp per (quarter, k-tile) covering both heads,
PV in bf16 with a ones-column producing softmax denominators, and per-
quarter normalization via reciprocal_approx_fast + gpsimd
partition_broadcast. Projection of pair p+1 is interleaved into pair p's
attention so the PE stays busy while ACT streams exps; the output
projection is interleaved into pair 3's attention.
"""

import numpy as np

B, S, D, H, DK = 4, 2048, 1024, 16, 64
THETA = 10000.0
N_CORES = 8
NKT = S // 128        # k tiles
NP = 4                # head pairs per core
NQ = 4                # q quarters of 512

_prog_cache = {}


def _apply_walrus_wait_workarounds():
    """This container's walrus rejects any TPB instruction with more than one
    sync wait. Patch the Tile kernel-tail drain to emit a chain of single-wait
    drains, and provide a post-pass that hoists excess waits onto NoOps."""
    import concourse.mybir as mybir
    import concourse.tile as tile_mod
    from concourse.vector_clock import ScopedClock

    def _drain_and_barrier(self, tick_clock, wait_clock):
        nc = self.nc
        drain_inst = nc.sync.drain()
        wait_clock.add_sem_waits(
            drain_inst.ins, ScopedClock({None: tick_clock.global_clock}))
        waits = list(drain_inst.ins.sync_info.on_wait)
        if len(waits) > 1:
            si = drain_inst.ins.sync_info
            si.on_wait = waits[:1]
            drain_inst.ins.sync_info = si
            for i in range(1, len(waits)):
                d2 = nc.sync.drain()
                d2.ins.sync_info = mybir.SyncInfo(
                    on_wait=waits[i:i + 1], on_update=[])
        nc.all_engine_barrier()
        popped = nc._tile_sem_poison_stack.pop()
        assert popped is self._sem_poison
        nc.clear_and_free_semaphores(list(self.sems.allocated().values()))
        nc.all_engine_barrier()

    tile_mod.TileContext._drain_and_barrier = _drain_and_barrier


def _split_waits(nc):
    import concourse.mybir as mybir
    engines = {mybir.EngineType.PE, mybir.EngineType.DVE, mybir.EngineType.SP,
               mybir.EngineType.Activation, mybir.EngineType.Pool}
    for f in nc.m.functions:
        for bb in f.blocks:
            out = []
            changed = False
            for ins in bb.instructions:
                si = ins.sync_info
                if si is not None and len(si.on_wait) > 1 and ins.engine in engines:
                    waits = list(si.on_wait)
                    for i in range(len(waits) - 1):
                        out.append(mybir.InstNoOp(
                            name=f"{ins.name}-waitsplit-{i}",
                            sync_info=mybir.SyncInfo(
                                on_wait=waits[i:i + 1], on_update=[]),
                            bass_nofuse=True, engine=ins.engine))
                    ins.sync_info = mybir.SyncInfo(
                        on_wait=waits[-1:], on_update=list(si.on_update))
                    changed = True
                out.append(ins)
            if changed:
                bb.instructions = out


def _pair_swap_mask():
    mask = []
    for j in range(16):
        mask += [2 * j + 1, 2 * j]
    return mask


def _build_program():
    _apply_walrus_wait_workarounds()
    import concourse.bass as bass
    import concourse.mybir as mybir
    import concourse.tile as tile
    from concourse import library_config
    from concourse.masks import make_identity
    from contextlib import ExitStack

    F32 = mybir.dt.float32
    F32R = mybir.dt.float32r
    BF16 = mybir.dt.bfloat16
    AF = mybir.ActivationFunctionType

    nc = bass.Bass()
    xb = nc.declare_dram_parameter("xb", [S, D], F32, isOutput=False)
    wqt = nc.declare_dram_parameter("wqt", [D, 512], F32, isOutput=False)
    wkt = nc.declare_dram_parameter("wkt", [D, 512], F32, isOutput=False)
    wvt = nc.declare_dram_parameter("wvt", [D, 512], F32, isOutput=False)
    wot = nc.declare_dram_parameter("wot", [512, D], F32, isOutput=False)
    cost = nc.declare_dram_parameter("cost", [128, S], F32, isOutput=False)
    sint2 = nc.declare_dram_parameter("sint2", [128, S], F32, isOutput=False)
    esel2 = nc.declare_dram_parameter("esel2", [8, 4, 128], F32, isOutput=False)
    y = nc.declare_dram_parameter("y", [S, D], F32, isOutput=True)

    swap_mask = _pair_swap_mask()

    with tile.TileContext(nc) as tc, ExitStack() as ctx:
        singles = ctx.enter_context(tc.tile_pool(name="singles", bufs=1))
        ident = singles.tile([128, 128], F32)
        make_identity(nc, ident)

        # persistent tensors
        xtc = singles.tile([128, 8, S], BF16, tag="xtc")       # x^T, bf16
        qslabs = [singles.tile([128, S], F32R, tag=f"qsl{p}", name=f"qsl{p}")
                  for p in range(NP)]                          # doubles as attn out
        vslab = singles.tile([128, NKT, 8, 65], BF16, tag="vslab")
        cosc = singles.tile([128, S], F32, tag="cosc")
        sinc = singles.tile([128, S], F32, tag="sinc")
        wvr = singles.tile([128, 8, 512], BF16, tag="wvr")
        wor = singles.tile([128, 4, D], F32R, tag="wor")
        selr = singles.tile([8, 4, 128], F32R, tag="selr")
        sums = [singles.tile([8, 512], F32, tag=f"sums{p}", name=f"sums{p}")
                for p in range(NP)]

        # pools
        xpool = ctx.enter_context(tc.tile_pool(name="xpool", bufs=4))
        wst = ctx.enter_context(tc.tile_pool(name="wst", bufs=2))
        wrq = ctx.enter_context(tc.tile_pool(name="wrq", bufs=2))
        wrk = ctx.enter_context(tc.tile_pool(name="wrk", bufs=2))
        kpool = ctx.enter_context(tc.tile_pool(name="kpool", bufs=2))
        ptpool = ctx.enter_context(tc.tile_pool(name="ptpool", bufs=4))
        tshp = ctx.enter_context(tc.tile_pool(name="tshp", bufs=2))
        nrm = ctx.enter_context(tc.tile_pool(name="nrm", bufs=1))
        ysb = ctx.enter_context(tc.tile_pool(name="ysb", bufs=2))
        ppp = ctx.enter_context(tc.tile_pool(name="ppp", bufs=2, space="PSUM"))
        pss = ctx.enter_context(tc.tile_pool(name="pss", bufs=2, space="PSUM"))
        posp = ctx.enter_context(tc.tile_pool(name="posp", bufs=1, space="PSUM"))

        nc.sync.dma_start(out=cosc, in_=cost[:, :])
        nc.sync.dma_start(out=sinc, in_=sint2[:, :])

        # ---- weight staging helpers ------------------------------------
        def stage_wv():
            wv_r = wvt.rearrange("(ic p) o -> p ic o", p=128)
            for ic in range(8):
                st = wst.tile([128, 1024], F32, tag="wst", name=f"wv{ic}")
                nc.sync.dma_start(out=st[:, 0:512], in_=wv_r[:, ic, :])
                nc.vector.tensor_copy(wvr[:, ic, :], st[:, 0:512])

        def stage_wqk(p):
            """DMA + cast pair p's q/k weight slices -> [128, 8, 128] bf16."""
            tiles = {}
            for name, src, pool in (("q", wqt, wrq), ("k", wkt, wrk)):
                src_r = src.rearrange("(ic pp) o -> pp ic o", pp=128)
                st = wst.tile([128, 1024], F32, tag="wst", name=f"w{name}st{p}")
                st_v = st.rearrange("pp (ic o) -> pp ic o", ic=8)
                nc.sync.dma_start(
                    out=st_v, in_=src_r[:, :, p * 128:(p + 1) * 128])
                wr = pool.tile([128, 8, 128], BF16, tag=f"wr{name}",
                               name=f"wr{name}{p}")
                nc.vector.tensor_copy(wr, st_v)
                tiles[name] = wr
            return tiles

        def stage_wo():
            wot_r = wot.rearrange("(ic p) o -> p ic o", p=128)
            for ic in range(4):
                st = wst.tile([128, 1024], F32, tag="wst", name=f"wo{ic}")
                nc.sync.dma_start(out=st, in_=wot_r[:, ic, :])
                nc.vector.tensor_copy(wor[:, ic, :], st)

        # ---- phase helpers ---------------------------------------------
        def emit_xt_chunk(sc4):
            """Transpose x s-chunk sc4 into xtc (bf16) + project v for it."""
            xts = []
            for ssub in range(4):
                xt = xpool.tile([128, D], F32, tag="x",
                                name=f"x{sc4}_{ssub}")
                s0 = sc4 * 512 + ssub * 128
                nc.sync.dma_start(out=xt, in_=xb[s0:s0 + 128, :])
                xts.append(xt)
            for ic in range(8):
                ptr = ppp.tile([128, 512], F32, tag="pp", name="ptr")
                for ssub in range(4):
                    nc.tensor.transpose(
                        ptr[:, ssub * 128:(ssub + 1) * 128],
                        xts[ssub][:, ic * 128:(ic + 1) * 128], ident)
                if sc4 == 0:
                    nc.scalar.copy(
                        out=xtc[:, ic, sc4 * 512:(sc4 + 1) * 512], in_=ptr)
                else:
                    nc.vector.tensor_copy(
                        xtc[:, ic, sc4 * 512:(sc4 + 1) * 512], ptr)
            # v projection for this s-chunk (all 8 heads)
            for ssub in range(4):
                pv = ppp.tile([128, 512], F32, tag="pp", name="pv")
                for ic in range(8):
                    nc.tensor.matmul(
                        pv,
                        lhsT=xtc[:, ic, sc4 * 512 + ssub * 128:
                                 sc4 * 512 + (ssub + 1) * 128],
                        rhs=wvr[:, ic, :],
                        start=(ic == 0), stop=(ic == 7))
                kt = sc4 * 4 + ssub
                nc.vector.tensor_copy(
                    vslab[:, kt, :, 0:64],
                    pv.rearrange("p (h dk) -> p h dk", h=8))

        def proj_qk_group(p, wr_tiles, t, sc4):
            """One projection group: pair p, tensor t in {q, k}, s-chunk sc4."""
            slab = qslabs[p] if t == "q" else kslab_tiles[p]
            wr = wr_tiles[t]
            ssl = slice(sc4 * 512, (sc4 + 1) * 512)
            pp = ppp.tile([128, 512], F32, tag="pp", name=f"pp{t}{p}_{sc4}")
            for ic in range(8):
                nc.tensor.matmul(
                    pp, lhsT=wr[:, ic, :], rhs=xtc[:, ic, ssl],
                    start=(ic == 0), stop=(ic == 7))
            tsh = tshp.tile([128, 512], F32, tag="tsh")
            nc.vector.stream_shuffle(tsh, pp, swap_mask)
            nc.gpsimd.tensor_mul(tsh, tsh, sinc[:, ssl])
            nc.vector.tensor_mul(slab[:, ssl], pp, cosc[:, ssl])
            nc.vector.tensor_add(slab[:, ssl], slab[:, ssl], tsh)

        kslab_tiles = {}

        def attention_quarter(p, q):
            """Attention for pair p, q-quarter q."""
            qsl0 = q * 512
            kslab = kslab_tiles[p]
            qslab = qslabs[p]
            pos = posp.tile([65, 1024], F32, tag="pos", name=f"pos{p}_{q}")
            njs = 4 * q + 4
            pending = []
            for j in range(njs):
                lo = max(0, 128 * j - 512 * q)
                w = 512 - lo
                qsl = slice(qsl0 + lo, qsl0 + 512)
                # head A scores at ps[:, 0:w] (bank 0), head B at
                # ps[:, 512:512+w] (bank 1) — each matmul stays in one bank
                ps = pss.tile([128, 1024], F32, tag="ps", name=f"ps{p}_{q}_{j}")
                nc.tensor.matmul(
                    ps[:, 0:w],
                    lhsT=kslab[0:64, j * 128:(j + 1) * 128],
                    rhs=qslab[0:64, qsl], start=True, stop=True)
                nc.tensor.matmul(
                    ps[:, 512:512 + w],
                    lhsT=kslab[64:128, j * 128:(j + 1) * 128],
                    rhs=qslab[64:128, qsl], start=True, stop=True)
                pt = ptpool.tile([128, 1024], BF16, tag="pt")
                nc.scalar.activation(out=pt[:, 0:512 + w],
                                     in_=ps[:, 0:512 + w],
                                     func=AF.Exp, scale=0.125)
                if j >= 4 * q:  # diagonal tile: mask upper triangle
                    for h in range(2):
                        nc.gpsimd.affine_select(
                            out=pt[:, h * 512:h * 512 + 128],
                            in_=pt[:, h * 512:h * 512 + 128],
                            compare_op=mybir.AluOpType.is_ge,
                            fill=0.0, base=0,
                            pattern=[[1, 128]], channel_multiplier=-1)
                pending.append((j, lo, w, pt))
                # drain PV one step behind exp
                if len(pending) > 1:
                    emit_pv(p, q, pos, *pending.pop(0), njs)
            while pending:
                emit_pv(p, q, pos, *pending.pop(0), njs)
            # ---- writeback (unnormalized; frees pos quickly) -----------
            stmp = nrm.tile([1, 1024], F32, tag="stmp")
            nc.vector.tensor_copy(stmp[0:1, 0:512], pos[64:65, 0:512])
            nc.vector.tensor_copy(stmp[0:1, 512:1024], pos[64:65, 512:1024])
            nc.sync.dma_start(out=sums[p][q:q + 1, :], in_=stmp[0:1, 0:512])
            nc.sync.dma_start(out=sums[p][4 + q:5 + q, :],
                              in_=stmp[0:1, 512:1024])
            qsl = slice(qsl0, qsl0 + 512)
            nc.vector.tensor_copy(qslab[0:64, qsl], pos[0:64, 0:512])
            nc.vector.tensor_copy(qslab[64:128, qsl], pos[0:64, 512:1024])

        def normalize_quarters(p, quarters):
            """Scale pair p's attention output by the softmax reciprocals for
            the given quarters (requires their sums rows to be final)."""
            qslab = qslabs[p]
            recs = nrm.tile([8, 512], F32, tag="recs")
            nc.vector.reciprocal(recs, sums[p])
            recr = nrm.tile([8, 512], F32R, tag="recr")
            nc.vector.tensor_copy(recr, recs)
            for q in quarters:
                pb = ppp.tile([128, 512], F32, tag="pp", name=f"pb{p}_{q}")
                nc.tensor.matmul(pb, lhsT=selr[:, q, :], rhs=recr,
                                 start=True, stop=True)
                qsl = slice(q * 512, (q + 1) * 512)
                nc.vector.tensor_mul(qslab[:, qsl], qslab[:, qsl], pb)

        def emit_pv(p, q, pos, j, lo, w, pt, njs):
            start = (j == 0)
            stop = (j == njs - 1)
            nc.tensor.matmul(
                pos[:, lo:512], lhsT=vslab[:, j, 2 * p, 0:65],
                rhs=pt[:, 0:w], start=start, stop=stop)
            nc.tensor.matmul(
                pos[:, 512 + lo:1024], lhsT=vslab[:, j, 2 * p + 1, 0:65],
                rhs=pt[:, 512:512 + w], start=start, stop=stop)

        def outproj_group(qs):
            for oh in range(2):
                py = ppp.tile([128, 512], F32, tag="pp", name=f"py{qs}_{oh}")
                for p in range(NP):
                    nc.tensor.matmul(
                        py, lhsT=qslabs[p][:, qs * 128:(qs + 1) * 128],
                        rhs=wor[:, p, oh * 512:(oh + 1) * 512],
                        start=(p == 0), stop=(p == 3))
                yt = ysb.tile([128, 512], F32, tag="yt", name=f"yt{qs}_{oh}")
                nc.scalar.copy(out=yt, in_=py)
                nc.sync.dma_start(
                    out=y[qs * 128:(qs + 1) * 128, oh * 512:(oh + 1) * 512],
                    in_=yt)

        # ================= emission =====================================
        selst = nrm.tile([8, 4, 128], F32, tag="selst")
        nc.sync.dma_start(out=selst, in_=esel2[:])
        nc.vector.tensor_copy(selr, selst)
        nc.vector.memset(vslab[:, :, :, 64:65], 1.0)
        nc.vector.memset(sums[NP - 1], 1.0)
        stage_wv()

        # global queue of projection groups, popped ahead of the attention
        # quarters that need them
        proj_plan = [(p, t, sc4) for p in range(NP)
                     for sc4 in range(4) for t in ("q", "k")]
        wr_by_pair = {}
        emitted = [0]

        def pop_proj(target):
            while emitted[0] < min(target, len(proj_plan)):
                p, t, sc4 = proj_plan[emitted[0]]
                if p not in wr_by_pair:
                    wr_by_pair[p] = stage_wqk(p)
                    kslab_tiles[p] = kpool.tile(
                        [128, S], F32R, tag="ks", name=f"ks{p}")
                proj_qk_group(p, wr_by_pair[p], t, sc4)
                emitted[0] += 1

        emit_xt_chunk(0)
        pop_proj(2)
        for sc4 in range(1, 4):
            emit_xt_chunk(sc4)
            pop_proj(2 * sc4 + 2)

        for p in range(NP):
            for q in range(NQ):
                # stay a full pair ahead of what pair p quarter q needs
                pop_proj(p * 8 + 2 * (q + 1) + 8)
                if p == 2 and q == 3:
                    stage_wo()
                if p == NP - 1:
                    # pairs 0-2 normalization + the output projection are
                    # interleaved into pair 3's attention quarters
                    if q == 0:
                        normalize_quarters(0, range(NQ))
                        normalize_quarters(1, range(NQ))
                    else:
                        if q == 1:
                            normalize_quarters(2, range(NQ))
                        normalize_quarters(p, [q - 1])
                        for qs in range((q - 1) * 4, q * 4):
                            outproj_group(qs)
                attention_quarter(p, q)
        normalize_quarters(NP - 1, [NQ - 1])
        for qs in range(12, 16):
            outproj_group(qs)

    _split_waits(nc)
    return nc


def _host_inputs(x, wq, wk, wv, wo, token_positions):
    pos = np.asarray(token_positions).astype(np.float64)
    ex = np.arange(0, DK, 2, dtype=np.float64) / DK
    freq = 1.0 / (THETA ** ex)
    f = pos[:, None] * freq[None, :]                       # [S, DK/2]
    cos = np.repeat(np.cos(f), 2, axis=1).astype(np.float32)   # [S, DK]
    sin = np.repeat(np.sin(f), 2, axis=1).astype(np.float32)
    cosT = np.ascontiguousarray(cos.T)                     # [DK, S]
    sinT = np.ascontiguousarray(sin.T)
    sgn = np.where(np.arange(DK) % 2 == 0, -1.0, 1.0).astype(np.float32)
    sinT2 = sinT * sgn[:, None]
    costile = np.tile(cosT, (2, 1))                        # [128, S]
    sintile = np.tile(sinT2, (2, 1))

    wqT = np.ascontiguousarray(wq.T)
    wkT = np.ascontiguousarray(wk.T)
    wvT = np.ascontiguousarray(wv.T)
    woT = np.ascontiguousarray(wo.T)

    # selector for the per-(quarter) reciprocal broadcast matmul:
    # pb[m] for quarter q picks sums row q (m<64, head A) or 4+q (head B)
    esel2 = np.zeros((8, 4, 128), np.float32)
    for q in range(4):
        esel2[q, q, 0:64] = 1.0
        esel2[4 + q, q, 64:128] = 1.0

    in_maps = []
    for core in range(N_CORES):
        b, hh = core // 2, core % 2
        osl = slice(hh * 512, (hh + 1) * 512)
        in_maps.append({
            "xb": np.ascontiguousarray(x[b]),
            "wqt": np.ascontiguousarray(wqT[:, osl]),
            "wkt": np.ascontiguousarray(wkT[:, osl]),
            "wvt": np.ascontiguousarray(wvT[:, osl]),
            "wot": np.ascontiguousarray(woT[osl, :]),
            "cost": costile,
            "sint2": sintile,
            "esel2": esel2,
        })
    return in_maps


def run_sharded(x, wq, wk, wv, wo, token_positions, trace=False):
    from concourse.bass_utils import run_bass_kernel_spmd
    if "nc" not in _prog_cache:
        _prog_cache["nc"] = _build_program()
    nc = _prog_cache["nc"]
    in_maps = _host_inputs(x, wq, wk, wv, wo, token_positions)
    res = run_bass_kernel_spmd(nc, in_maps, list(range(N_CORES)), trace=trace)
    out = np.empty((B, S, D), np.float32)
    for b in range(B):
        out[b] = res.results[2 * b]["y"] + res.results[2 * b + 1]["y"]
    return out, res


def kernel(x, wq, wk, wv, wo, token_positions):
    x = np.asarray(x, dtype=np.float32)
    out, _ = run_sharded(
        x, np.asarray(wq, np.float32), np.asarray(wk, np.float32),
        np.asarray(wv, np.float32), np.asarray(wo, np.float32),
        np.asarray(token_positions))
    return out
